# revision 21
# baseline (speedup 1.0000x reference)
"""Trainium2 Bass kernel for causal MultiHeadAttention + residual + LayerNorm.

Problem: nn_MultiHeadAttention_88124138979383
  B=2, L=2048, D=1024, H=16, DH=64, causal mask, out-proj, residual, LN.

Sharding (8 cores): core c = (batch b=c//4, head-group g=c%4, 4 heads each).
Each core projects Q^T/K^T (head-dim on partitions) and V (natural layout)
for its batch+heads, runs causal attention in scores-transposed layout
(softmax reduction via a ones-column appended to V inside the P@V matmul,
no max-subtraction — scores are small), normalizes via a GPSIMD partition-
broadcast of the reciprocal denominator, then an 8-core AllToAll exchanges
head-shards for sequence-shards per q-block: after AllToAll #qb, core c
has the full 16-head A^T for queries [512qb+64c : +64] of BOTH batches and
computes the output projection, residual (query rows + bo pre-added on
host) and LayerNorm for those 128 rows — out-proj #qb is issued one block
late so its gather DMA's collective wait is already satisfied and never
head-of-line-blocks the SP DMA queue.

Causal masking is done on the PE: a small extra matmul accumulates -240
into the masked upper-triangle band of the diagonal score tiles, so
exp() drives masked entries to 0 (no DVE mask multiply needed).
LayerNorm's 1/sqrt(var+eps) is computed as exp(-0.5*log(var+eps)) so the
scalar engine only ever needs the exp/log table set (one load, no thrash).

All matmuls in bf16 (fp32 PSUM accumulate). Host transposes/casts inputs.
"""
import os
import sys

for _p in ("/opt/trn_rl_repo", os.path.join(os.path.expanduser("~"), ".axon_site", "_ro", "trn_rl_repo")):
    if os.path.isdir(_p) and _p not in sys.path:
        sys.path.insert(0, _p)

import numpy as np
import ml_dtypes

import concourse.bass as bass
import concourse.tile as tile
from concourse import bacc, mybir
from concourse.hw_specs import get_activation_tables as _real_gat


def _gat_prefer_combined(arch):
    """Table-set view where Exp/Ln are only satisfiable by the combined
    natural_log_exp_and_others set, so the ACT table is loaded once instead
    of thrashing between exp- and ln-anchored sets on every LayerNorm.
    Entries/order (= act_func_set_id) are unchanged; runtime tables are the
    real ones, so numerics are unaffected."""
    full = _real_gat(arch)
    exp_f = mybir.ActivationFunctionType.Exp
    ln_f = mybir.ActivationFunctionType.Ln
    out = {}
    for name, funcs in full.items():
        f = set(funcs)
        if name != "natural_log_exp_and_others":
            f.discard(exp_f)
            f.discard(ln_f)
        out[name] = f
    return out


bacc.get_activation_tables = _gat_prefer_combined

BF16 = ml_dtypes.bfloat16
F32 = mybir.dt.float32
BF = mybir.dt.bfloat16

B, L, D = 2, 2048, 1024
H, DH = 16, 64
NCORES = 8
HL = 4                 # heads per core
NPAIR = 2              # head pairs per core
MBS = 512              # m-block size for projections
NMB = L // MBS         # 4
QBS = 512              # q-block size for attention
NQB = L // QBS         # 4
KTS = 128              # k-tile size
NKT = L // KTS         # 16
MS = L // NCORES       # 256: per-core row count for out-proj/LN (both batches)
LN_EPS = 1e-5
SCALE = 1.0 / 8.0      # 1/sqrt(DH)
NEG = -240.0           # causal mask additive constant (exp(NEG/8) ~= 0)


def build_nc(reps: int = 1, phases: str = 'full', with_biases: bool = False):
    nc = bacc.Bacc("TRN2", target_bir_lowering=False, debug=False, num_devices=NCORES)
    qT = nc.dram_tensor("qT", [D, L], BF, kind="ExternalInput")
    kT = nc.dram_tensor("kT", [D, L], BF, kind="ExternalInput")
    vT = nc.dram_tensor("vT", [D, L], BF, kind="ExternalInput")
    wq = nc.dram_tensor("wq", [D, HL * DH], BF, kind="ExternalInput")
    wk = nc.dram_tensor("wk", [D, HL * DH], BF, kind="ExternalInput")
    wv = nc.dram_tensor("wv", [D, HL * DH], BF, kind="ExternalInput")
    wo = nc.dram_tensor("wo", [D, D], BF, kind="ExternalInput")
    bq = nc.dram_tensor("bq", [HL * DH, 1], F32, kind="ExternalInput")
    bk = nc.dram_tensor("bk", [HL * DH, 1], F32, kind="ExternalInput")
    bv = nc.dram_tensor("bv", [DH, HL], F32, kind="ExternalInput")
    qresbo = nc.dram_tensor("qresbo", [2 * MS, D], F32, kind="ExternalInput")
    gamma = nc.dram_tensor("gamma", [1, D], F32, kind="ExternalInput")
    beta = nc.dram_tensor("beta", [1, D], F32, kind="ExternalInput")
    y = nc.dram_tensor("y", [2 * MS, D], F32, kind="ExternalOutput")

    with tile.TileContext(nc) as tc:
        with (
            tc.tile_pool(name="consts", bufs=1) as consts,
            tc.tile_pool(name="persist", bufs=1) as persist,
            tc.tile_pool(name="xin", bufs=2) as xin,
            tc.tile_pool(name="es", bufs=6) as espool,
            tc.tile_pool(name="work", bufs=2) as work,
            tc.tile_pool(name="ps_proj", bufs=2, space="PSUM") as ps_proj,
            tc.tile_pool(name="ps_s", bufs=2, space="PSUM") as ps_s,
            tc.tile_pool(name="ps_acc", bufs=2, space="PSUM") as ps_acc,
            tc.tile_pool(name="dram", bufs=1, space="DRAM") as dram,
        ):
            # ---- constants / weights ----
            wq_sb = consts.tile([128, 8, HL * DH], BF, tag="wq")
            wk_sb = consts.tile([128, 8, HL * DH], BF, tag="wk")
            wv_sb = consts.tile([128, 8, HL * DH], BF, tag="wv")
            for hf in range(2):
                nc.sync.dma_start(out=wq_sb[:, 4 * hf:4 * hf + 4, :],
                                  in_=wq[512 * hf:512 * hf + 512, :].rearrange("(t p) n -> p t n", p=128))

            def load_wkv():
                nc.sync.dma_start(out=wk_sb, in_=wk.rearrange("(t p) n -> p t n", p=128))
                nc.sync.dma_start(out=wv_sb, in_=wv.rearrange("(t p) n -> p t n", p=128))
            wo_sb = consts.tile([128, 8, D], BF, tag="wo")
            bq_sb = consts.tile([128, NPAIR], F32, tag="bq")
            bk_sb = consts.tile([128, NPAIR], F32, tag="bk")
            nc.sync.dma_start(out=bq_sb, in_=bq.rearrange("(t p) o -> p (t o)", p=128))
            nc.sync.dma_start(out=bk_sb, in_=bk.rearrange("(t p) o -> p (t o)", p=128))
            bv_sb = consts.tile([DH, HL], F32, tag="bv")
            nc.sync.dma_start(out=bv_sb, in_=bv[:, :])
            eps_sb = consts.tile([128, 1], F32, tag="eps")
            nc.gpsimd.memset(eps_sb, LN_EPS)
            # causal-mask matmul constants: negT[r, c] = NEG where r < c else 0
            # (strictly upper); ident = I. PE adds into the diagonal band of a
            # score tile: s[p, q'] += sum_k negT[k, p]*I[k, q'] = NEG where q'<p.
            negT_sb = consts.tile([128, KTS], BF, tag="negT")
            nc.gpsimd.memset(negT_sb, NEG)
            nc.gpsimd.affine_select(
                out=negT_sb, in_=negT_sb,
                compare_op=mybir.AluOpType.is_ge, fill=0.0,
                base=-1, channel_multiplier=-1, pattern=[[1, KTS]])
            ident_sb = consts.tile([128, KTS], BF, tag="ident")
            nc.gpsimd.memset(ident_sb, 1.0)
            nc.gpsimd.affine_select(
                out=ident_sb, in_=ident_sb,
                compare_op=mybir.AluOpType.is_ge, fill=0.0,
                base=0, channel_multiplier=-1, pattern=[[1, KTS]])
            nc.gpsimd.affine_select(
                out=ident_sb, in_=ident_sb,
                compare_op=mybir.AluOpType.is_ge, fill=0.0,
                base=0, channel_multiplier=1, pattern=[[-1, KTS]])

            gam_sb = consts.tile([128, D], F32, tag="gam")
            bet_sb = consts.tile([128, D], F32, tag="bet")
            qres_sb4 = consts.tile([128, NQB, D], F32, tag="qres4")

            # ---- persistent activations ----
            qT_sb = [persist.tile([128, L], BF, tag=f"qT{p}", name=f"qT_sb{p}") for p in range(NPAIR)]
            kT_sb = [persist.tile([128, L], BF, tag=f"kT{p}", name=f"kT_sb{p}") for p in range(NPAIR)]
            # V in natural [seq, d] layout, 128 cols per head: col 0 = ones
            # (softmax denominator lands at PSUM partition 0, where GPSIMD
            # partition_broadcast can read it), cols 1:64 zero pad, 64:128 = V
            # (A rows land at partitions 64:128; DVE partition-shifts them to
            # 0:64 during the normalize multiply)
            v128_sb = persist.tile([128, NKT, HL * 128], BF, tag="v128")
            nc.gpsimd.memset(v128_sb, 0.0)
            nc.gpsimd.memset(
                v128_sb.rearrange("p kt (h x) -> p kt h x", x=128)[:, :, :, 0:1], 1.0)
            # normalized attention output A^T: [DH, head, L]
            a4_sb = persist.tile([DH, HL, L], BF, tag="a4", name="a4_sb")
            # gathered A^T after per-qb A2A: [part, qb, ctile, batch, m]
            ob2_sb = persist.tile([128, NQB, 8, 2, DH], BF, tag="ob2", name="ob2_sb")

            in_bq = [dram.tile([L, DH], BF, name=f"in_bq{i}") for i in range(NQB)]
            out_bq = [dram.tile([L, DH], BF, name=f"out_bq{i}") for i in range(NQB)]

            xin_tiles = {}

            def issue_loads(mb, split_in=False):
                m0 = mb * MBS
                xq = xin.tile([128, 8, MBS], BF, tag="xq", name=f"xq{mb}")
                xk = xin.tile([128, 8, MBS], BF, tag="xk", name=f"xk{mb}")
                xv = xin.tile([128, 8, MBS], BF, tag="xv", name=f"xv{mb}")
                xin_tiles[mb] = (xq, xk, xv)
                if split_in:
                    # halve the first transfers so the first matmuls start sooner;
                    # wk/wv load after xq (needed only once Q's matmuls are running)
                    for hf in range(2):
                        nc.sync.dma_start(
                            out=xq[:, 4 * hf:4 * hf + 4, :],
                            in_=qT[512 * hf:512 * hf + 512, m0:m0 + MBS].rearrange(
                                "(t p) m -> p t m", p=128))
                    load_wkv()
                    for src_t, dst in ((kT, xk), (vT, xv)):
                        for hf in range(2):
                            nc.sync.dma_start(
                                out=dst[:, 4 * hf:4 * hf + 4, :],
                                in_=src_t[512 * hf:512 * hf + 512, m0:m0 + MBS].rearrange(
                                    "(t p) m -> p t m", p=128))
                else:
                    nc.sync.dma_start(out=xq, in_=qT[:, m0:m0 + MBS].rearrange("(t p) m -> p t m", p=128))
                    nc.sync.dma_start(out=xk, in_=kT[:, m0:m0 + MBS].rearrange("(t p) m -> p t m", p=128))
                    nc.sync.dma_start(out=xv, in_=vT[:, m0:m0 + MBS].rearrange("(t p) m -> p t m", p=128))

            def proj_compute(mb):
                m0 = mb * MBS
                xq, xk, xv = xin_tiles.pop(mb)
                for p in range(NPAIR):
                    psq = ps_proj.tile([128, MBS], F32, tag="proj")
                    for t in range(8):
                        nc.tensor.matmul(psq[:], wq_sb[:, t, 128 * p:128 * p + 128], xq[:, t, :],
                                         start=(t == 0), stop=(t == 7))
                    if with_biases:
                        nc.vector.tensor_scalar_add(qT_sb[p][:, m0:m0 + MBS], psq[:], bq_sb[:, p:p + 1])
                    else:
                        nc.vector.tensor_copy(qT_sb[p][:, m0:m0 + MBS], psq[:])
                    psk = ps_proj.tile([128, MBS], F32, tag="proj")
                    for t in range(8):
                        nc.tensor.matmul(psk[:], wk_sb[:, t, 128 * p:128 * p + 128], xk[:, t, :],
                                         start=(t == 0), stop=(t == 7))
                    if with_biases:
                        nc.vector.tensor_scalar_add(kT_sb[p][:, m0:m0 + MBS], psk[:], bk_sb[:, p:p + 1])
                    else:
                        nc.vector.tensor_copy(kT_sb[p][:, m0:m0 + MBS], psk[:])
                for ms in range(MBS // 128):
                    mt = mb * (MBS // 128) + ms
                    psv = ps_proj.tile([128, HL * DH], F32, tag="proj")
                    for t in range(8):
                        nc.tensor.matmul(psv[:], xv[:, t, 128 * ms:128 * ms + 128], wv_sb[:, t, :],
                                         start=(t == 0), stop=(t == 7))
                    # write into v128 slots (strided dest); bv is added post-normalize
                    dst = v128_sb[:, mt, :].rearrange("p (h x) -> p h x", x=128)[:, :, 64:128]
                    nc.vector.tensor_copy(dst, psv[:].rearrange("p (h x) -> p h x", x=DH))

            def attn_block(qb, do_a2a=True):
                q0 = qb * QBS
                nkt = 4 * qb + 4
                for p in range(NPAIR):
                    at_e = ps_acc.tile([128, QBS], F32, tag="acc", name="at_e")
                    at_o = ps_acc.tile([128, QBS], F32, tag="acc", name="at_o")
                    es_prev = None
                    for kt in range(nkt):
                        k0 = kt * KTS
                        s = ps_s.tile([128, 2, QBS], F32, tag="s")
                        d = kt - 4 * qb
                        diag = d >= 0
                        # causally-valid q-slice of this tile (cols < off are fully masked)
                        off = 128 * d if d > 0 else 0
                        nc.tensor.matmul(s[:, 0, :], kT_sb[p][0:64, k0:k0 + KTS],
                                         qT_sb[p][0:64, q0:q0 + QBS],
                                         start=True, stop=not diag)
                        nc.tensor.matmul(s[:, 1, :], kT_sb[p][64:128, k0:k0 + KTS],
                                         qT_sb[p][64:128, q0:q0 + QBS],
                                         start=True, stop=not diag)
                        if diag:  # diagonal-crossing tile: add NEG to masked band
                            b0 = 128 * d
                            for lane in range(2):
                                nc.tensor.matmul(s[:, lane, b0:b0 + KTS], negT_sb, ident_sb,
                                                 start=False, stop=True)
                        es = espool.tile([128, 2, QBS], BF, tag="es")
                        nc.scalar.activation(out=es[:, :, off:], in_=s[:, :, off:],
                                             func=mybir.ActivationFunctionType.Exp, scale=SCALE)
                        # PV for previous kt was already emitted; emit this kt's PV now.
                        # (software pipeline: scores of kt+1 queue ahead of PV of kt on PE)
                        if es_prev is not None:
                            pkt, poff, pes = es_prev
                            nc.tensor.matmul(at_e[:, poff:], v128_sb[:, pkt, 128 * 2 * p:128 * 2 * p + 128],
                                             pes[:, 0, poff:], start=(pkt == 0), stop=False)
                            nc.tensor.matmul(at_o[:, poff:], v128_sb[:, pkt, 128 * (2 * p + 1):128 * (2 * p + 1) + 128],
                                             pes[:, 1, poff:], start=(pkt == 0), stop=False)
                        es_prev = (kt, off, es)
                    pkt, poff, pes = es_prev
                    nc.tensor.matmul(at_e[:, poff:], v128_sb[:, pkt, 128 * 2 * p:128 * 2 * p + 128],
                                     pes[:, 0, poff:], start=(pkt == 0), stop=True)
                    nc.tensor.matmul(at_o[:, poff:], v128_sb[:, pkt, 128 * (2 * p + 1):128 * (2 * p + 1) + 128],
                                     pes[:, 1, poff:], start=(pkt == 0), stop=True)
                    # normalize: A = A_unnorm * (1/colsum); den sits at PSUM
                    # partition 0 (GPSIMD pbcast reads only partition 0), A rows
                    # at 64:128 (DVE legally partition-shifts 64:128 -> 0:64)
                    for par, at in ((0, at_e), (1, at_o)):
                        h = 2 * p + par
                        rec = work.tile([1, QBS], BF, tag="rec")
                        with nc.allow_low_precision("bf16 softmax reciprocal is within tolerance"):
                            nc.vector.reciprocal(out=rec[:, :], in_=at[0:1, :])
                        bc_sb = work.tile([64, QBS], BF, tag="bc_sb")
                        nc.gpsimd.partition_broadcast(bc_sb[:, :], rec[:, :])
                        nc.vector.tensor_mul(a4_sb[:, h, q0:q0 + QBS], at[64:128, :], bc_sb[:])
                        if with_biases:
                            nc.vector.tensor_scalar_add(a4_sb[:, h, q0:q0 + QBS],
                                                        a4_sb[:, h, q0:q0 + QBS],
                                                        bv_sb[:, h:h + 1])
                if not do_a2a:
                    return
                # A2A input for this q-block: dest chunk j gets A^T cols
                # [512qb+64j : +64] in [(h p) m] row layout
                for h in range(HL):
                    nc.sync.dma_start(
                        out=in_bq[qb].rearrange("(j h p) m -> p h j m", j=NCORES, h=HL, p=DH)[:, h],
                        in_=a4_sb[:, h, q0:q0 + QBS].rearrange("p (j m) -> p j m", j=NCORES))
                nc.gpsimd.collective_compute(
                    "AllToAll", mybir.AluOpType.bypass,
                    ins=[in_bq[qb].opt()], outs=[out_bq[qb].opt()],
                    replica_groups=[list(range(NCORES))])

            def outproj_block(qb):
                # gather so that ctile t has batch0 (rows of src cores 0-3) in
                # cols 0:64 and batch1 in cols 64:128 (same (h,dh) rows); this
                # DMA waits on AllToAll #qb — issued one block late so the wait
                # is (nearly) satisfied at queue-head time
                for b_ in range(2):
                    nc.sync.dma_start(
                        out=ob2_sb[:, qb, :, b_, :],
                        in_=out_bq[qb].rearrange("(b t p) m -> p t b m", b=2, t=8, p=128)[:, :, b_])
                # 128 rows: queries [512qb+64c : +64] of batch0 then batch1
                x_sb = work.tile([128, D], F32, tag="x")
                for nb in range(2):
                    o_ps = ps_proj.tile([128, 512], F32, tag="proj", name="o_ps")
                    for t in range(8):
                        nc.tensor.matmul(o_ps[:],
                                         ob2_sb[:, qb, t, :, :],
                                         wo_sb[:, t, 512 * nb:512 * nb + 512],
                                         start=(t == 0), stop=(t == 7))
                    nc.vector.tensor_add(x_sb[:, 512 * nb:512 * nb + 512], o_ps[:],
                                         qres_sb4[:, qb, 512 * nb:512 * nb + 512])
                stats = work.tile([128, 2, 6], F32, tag="stats")
                nc.vector.bn_stats(out=stats[:, 0, :], in_=x_sb[:, 0:512])
                nc.vector.bn_stats(out=stats[:, 1, :], in_=x_sb[:, 512:1024])
                mv = work.tile([128, 2], F32, tag="mv")
                nc.vector.bn_aggr(out=mv[:], in_=stats[:])
                # rstd = 1/sqrt(var+eps) = exp(-0.5*log(var+eps)): stays inside
                # the exp/log ACT table set (no table switch)
                rstd = work.tile([128, 1], F32, tag="rstd")
                nc.scalar.activation(out=rstd[:], in_=mv[:, 1:2],
                                     func=mybir.ActivationFunctionType.Ln,
                                     bias=eps_sb[:, 0:1], scale=1.0)
                nc.scalar.activation(out=rstd[:], in_=rstd[:],
                                     func=mybir.ActivationFunctionType.Exp, scale=-0.5)
                y_sb = work.tile([128, D], F32, tag="y")
                nc.vector.tensor_scalar(out=y_sb[:], in0=x_sb[:],
                                        scalar1=mv[:, 0:1], scalar2=rstd[:, 0:1],
                                        op0=mybir.AluOpType.subtract,
                                        op1=mybir.AluOpType.mult)
                nc.vector.scalar_tensor_tensor(out=y_sb[:], in0=y_sb[:], scalar=1.0,
                                               in1=gam_sb[:],
                                               op0=mybir.AluOpType.mult,
                                               op1=mybir.AluOpType.mult)
                nc.vector.tensor_add(y_sb[:], y_sb[:], bet_sb[:])
                nc.sync.dma_start(out=y[128 * qb:128 * qb + 128, :], in_=y_sb[:])

            for _rep in range(reps):
              for i in range(NMB):
                if i == 0:
                    issue_loads(0, split_in=(_rep == 0))
                proj_compute(i)
                if i + 1 < NMB:
                    issue_loads(i + 1)
                if _rep == 0 and i == 1:
                    # E-phase constants: issued after xin(2) so they don't
                    # delay the projection pipeline; needed from out-proj #0 on
                    nc.sync.dma_start(out=wo_sb, in_=wo.rearrange("(t p) n -> p t n", p=128))
                    nc.sync.dma_start(out=qres_sb4,
                                      in_=qresbo.rearrange("(r p) n -> p r n", p=128))
                    nc.sync.dma_start(out=gam_sb, in_=gamma[:, :].to_broadcast([128, D]))
                    nc.sync.dma_start(out=bet_sb, in_=beta[:, :].to_broadcast([128, D]))
                if phases == 'full' and i >= 1:
                    outproj_block(i - 1)
                if phases != 'proj':
                    attn_block(i, do_a2a=(phases in ('a2a', 'full')))
              if phases == 'full':
                  outproj_block(NMB - 1)
    nc.finalize()
    return nc


_CACHE = {}


def _prep_inputs(query, key, value, Wq, bq, Wk, bk, Wv, bv, Wo, bo, gamma, beta):
    """Host-side shard + transpose + cast. Returns per-core in_maps."""
    q32 = np.asarray(query, np.float32)
    qT = [np.ascontiguousarray(q32[b].T).astype(BF16) for b in range(B)]
    kTt = [np.ascontiguousarray(np.asarray(key, np.float32)[b].T).astype(BF16) for b in range(B)]
    vTt = [np.ascontiguousarray(np.asarray(value, np.float32)[b].T).astype(BF16) for b in range(B)]
    Wqb = np.asarray(Wq, np.float32).astype(BF16)
    Wkb = np.asarray(Wk, np.float32).astype(BF16)
    Wvb = np.asarray(Wv, np.float32).astype(BF16)
    Wob = np.ascontiguousarray(np.asarray(Wo, np.float32)).astype(BF16)
    bo32 = np.asarray(bo, np.float32)
    in_maps = []
    for c in range(NCORES):
        b, g = divmod(c, 4)
        sl = slice(HL * DH * g, HL * DH * (g + 1))
        qres = np.concatenate(
            [q32[b_, 512 * qb + 64 * c: 512 * qb + 64 * c + 64] + bo32
             for qb in range(4) for b_ in range(B)], axis=0)
        in_maps.append({
            "qT": qT[b], "kT": kTt[b], "vT": vTt[b],
            "wq": np.ascontiguousarray(Wqb[:, sl]),
            "wk": np.ascontiguousarray(Wkb[:, sl]),
            "wv": np.ascontiguousarray(Wvb[:, sl]),
            "wo": Wob,
            "bq": np.ascontiguousarray(np.asarray(bq, np.float32)[sl]).reshape(HL * DH, 1),
            "bk": np.ascontiguousarray(np.asarray(bk, np.float32)[sl]).reshape(HL * DH, 1),
            "bv": np.ascontiguousarray(np.asarray(bv, np.float32)[sl].reshape(HL, DH).T),
            "qresbo": np.ascontiguousarray(qres, np.float32),
            "gamma": np.asarray(gamma, np.float32).reshape(1, D),
            "beta": np.asarray(beta, np.float32).reshape(1, D),
        })
    return in_maps


def _assemble(results):
    out = np.empty((B, L, D), np.float32)
    for c in range(NCORES):
        yc = results[c]["y"]
        for b_ in range(B):
            for qb in range(4):
                out[b_, 512 * qb + 64 * c: 512 * qb + 64 * c + 64] = \
                    yc[128 * qb + 64 * b_: 128 * qb + 64 * b_ + 64]
    return out


def kernel(**inputs) -> np.ndarray:
    from concourse.bass_utils import run_bass_kernel_spmd
    in_maps = _prep_inputs(
        inputs["query"], inputs["key"], inputs["value"],
        inputs["Wq"], inputs["bq"], inputs["Wk"], inputs["bk"],
        inputs["Wv"], inputs["bv"], inputs["Wo"], inputs["bo"],
        inputs["gamma"], inputs["beta"])
    wb = any(np.any(np.asarray(inputs[k]) != 0) for k in ("bq", "bk", "bv"))
    key = ("nc", wb)
    if key not in _CACHE:
        _CACHE[key] = build_nc(with_biases=wb)
    _CACHE["nc"] = _CACHE[key]
    res = run_bass_kernel_spmd(_CACHE[key], in_maps, core_ids=list(range(NCORES)))
    return _assemble(res.results)


if __name__ == "__main__":
    # quick shape check of the program build
    nc = build_nc()
    n_inst = sum(len(bb.instructions) for f in nc.m.functions for bb in f.blocks)
    print("built ok, instructions:", n_inst)


# revision 25
# speedup vs baseline: 4.2500x; 4.2500x over previous
"""Trainium2 Bass kernel for causal MultiHeadAttention + residual + LayerNorm.

Problem: nn_MultiHeadAttention_88124138979383
  B=2, L=2048, D=1024, H=16, DH=64, causal mask, out-proj, residual, LN.

Sharding (8 cores): core c = (batch b=c//4, head-group g=c%4, 4 heads each).
Each core projects Q^T/K^T (head-dim on partitions) and V (natural layout)
for its batch+heads, runs causal attention in scores-transposed layout
(softmax reduction via a ones-column appended to V inside the P@V matmul,
no max-subtraction — scores are small), normalizes via a GPSIMD partition-
broadcast of the reciprocal denominator, then an 8-core AllToAll exchanges
head-shards for sequence-shards per q-block: after AllToAll #qb, core c
has the full 16-head A^T for queries [512qb+64c : +64] of BOTH batches and
computes the output projection, residual (query rows + bo pre-added on
host) and LayerNorm for those 128 rows — out-proj #qb is issued one block
late so its gather DMA's collective wait is already satisfied and never
head-of-line-blocks the SP DMA queue.

Causal masking is done on the PE: a small extra matmul accumulates -240
into the masked upper-triangle band of the diagonal score tiles, so
exp() drives masked entries to 0 (no DVE mask multiply needed).
LayerNorm's 1/sqrt(var+eps) is computed as exp(-0.5*log(var+eps)) so the
scalar engine only ever needs the exp/log table set (one load, no thrash).

All matmuls in bf16 (fp32 PSUM accumulate). Host transposes/casts inputs.
"""
import os
import sys

for _p in ("/opt/trn_rl_repo", os.path.join(os.path.expanduser("~"), ".axon_site", "_ro", "trn_rl_repo")):
    if os.path.isdir(_p) and _p not in sys.path:
        sys.path.insert(0, _p)

import numpy as np
import ml_dtypes

import concourse.bass as bass
import concourse.tile as tile
from concourse import bacc, mybir
from concourse.hw_specs import get_activation_tables as _real_gat


def _gat_prefer_combined(arch):
    """Table-set view where Exp/Ln are only satisfiable by the combined
    natural_log_exp_and_others set, so the ACT table is loaded once instead
    of thrashing between exp- and ln-anchored sets on every LayerNorm.
    Entries/order (= act_func_set_id) are unchanged; runtime tables are the
    real ones, so numerics are unaffected."""
    full = _real_gat(arch)
    exp_f = mybir.ActivationFunctionType.Exp
    ln_f = mybir.ActivationFunctionType.Ln
    out = {}
    for name, funcs in full.items():
        f = set(funcs)
        if name != "natural_log_exp_and_others":
            f.discard(exp_f)
            f.discard(ln_f)
        out[name] = f
    return out


bacc.get_activation_tables = _gat_prefer_combined

BF16 = ml_dtypes.bfloat16
F32 = mybir.dt.float32
BF = mybir.dt.bfloat16

B, L, D = 2, 2048, 1024
H, DH = 16, 64
NCORES = 8
HL = 4                 # heads per core
NPAIR = 2              # head pairs per core
MBS = 512              # m-block size for projections
NMB = L // MBS         # 4
QBS = 512              # q-block size for attention
NQB = L // QBS         # 4
KTS = 128              # k-tile size
NKT = L // KTS         # 16
MS = L // NCORES       # 256: per-core row count for out-proj/LN (both batches)
LN_EPS = 1e-5
SCALE = 1.0 / 8.0      # 1/sqrt(DH)
NEG = -240.0           # causal mask additive constant (exp(NEG/8) ~= 0)


def build_nc(reps: int = 1, phases: str = 'full', with_biases: bool = False):
    nc = bacc.Bacc("TRN2", target_bir_lowering=False, debug=False, num_devices=NCORES)
    qT = nc.dram_tensor("qT", [D, L], BF, kind="ExternalInput")
    kT = nc.dram_tensor("kT", [D, L], BF, kind="ExternalInput")
    vT = nc.dram_tensor("vT", [D, L], BF, kind="ExternalInput")
    wq = nc.dram_tensor("wq", [D, HL * DH], BF, kind="ExternalInput")
    wk = nc.dram_tensor("wk", [D, HL * DH], BF, kind="ExternalInput")
    wv = nc.dram_tensor("wv", [D, HL * DH], BF, kind="ExternalInput")
    wo = nc.dram_tensor("wo", [D, D], BF, kind="ExternalInput")
    bq = nc.dram_tensor("bq", [HL * DH, 1], F32, kind="ExternalInput")
    bk = nc.dram_tensor("bk", [HL * DH, 1], F32, kind="ExternalInput")
    bv = nc.dram_tensor("bv", [DH, HL], F32, kind="ExternalInput")
    qresbo = nc.dram_tensor("qresbo", [2 * MS, D], F32, kind="ExternalInput")
    gamma = nc.dram_tensor("gamma", [1, D], F32, kind="ExternalInput")
    beta = nc.dram_tensor("beta", [1, D], F32, kind="ExternalInput")
    y = nc.dram_tensor("y", [2 * MS, D], F32, kind="ExternalOutput")

    with tile.TileContext(nc) as tc:
        with (
            tc.tile_pool(name="consts", bufs=1) as consts,
            tc.tile_pool(name="persist", bufs=1) as persist,
            tc.tile_pool(name="xin", bufs=2) as xin,
            tc.tile_pool(name="es", bufs=8) as espool,
            tc.tile_pool(name="work", bufs=2) as work,
            tc.tile_pool(name="ps_proj", bufs=2, space="PSUM") as ps_proj,
            tc.tile_pool(name="ps_s", bufs=3, space="PSUM") as ps_s,
            tc.tile_pool(name="ps_acc", bufs=2, space="PSUM") as ps_acc,
            tc.tile_pool(name="ps_out", bufs=1, space="PSUM") as ps_out,
            tc.tile_pool(name="dram", bufs=1, space="DRAM") as dram,
        ):
            # ---- constants / weights ----
            wq_sb = consts.tile([128, 8, HL * DH], BF, tag="wq")
            wk_sb = consts.tile([128, 8, HL * DH], BF, tag="wk")
            wv_sb = consts.tile([128, 8, HL * DH], BF, tag="wv")
            for hf in range(2):
                nc.sync.dma_start(out=wq_sb[:, 4 * hf:4 * hf + 4, :],
                                  in_=wq[512 * hf:512 * hf + 512, :].rearrange("(t p) n -> p t n", p=128))

            def load_wkv():
                nc.sync.dma_start(out=wk_sb, in_=wk.rearrange("(t p) n -> p t n", p=128))
                nc.sync.dma_start(out=wv_sb, in_=wv.rearrange("(t p) n -> p t n", p=128))
            wo_sb = consts.tile([128, 8, D], BF, tag="wo")
            bq_sb = consts.tile([128, NPAIR], F32, tag="bq")
            bk_sb = consts.tile([128, NPAIR], F32, tag="bk")
            nc.sync.dma_start(out=bq_sb, in_=bq.rearrange("(t p) o -> p (t o)", p=128))
            nc.sync.dma_start(out=bk_sb, in_=bk.rearrange("(t p) o -> p (t o)", p=128))
            bv_sb = consts.tile([DH, HL], F32, tag="bv")
            nc.sync.dma_start(out=bv_sb, in_=bv[:, :])
            eps_sb = consts.tile([128, 1], F32, tag="eps")
            nc.gpsimd.memset(eps_sb, LN_EPS)
            # causal-mask matmul constants: negT[r, c] = NEG where r < c else 0
            # (strictly upper); ident = I. PE adds into the diagonal band of a
            # score tile: s[p, q'] += sum_k negT[k, p]*I[k, q'] = NEG where q'<p.
            negT_sb = consts.tile([128, KTS], BF, tag="negT")
            nc.gpsimd.memset(negT_sb, NEG)
            nc.gpsimd.affine_select(
                out=negT_sb, in_=negT_sb,
                compare_op=mybir.AluOpType.is_ge, fill=0.0,
                base=-1, channel_multiplier=-1, pattern=[[1, KTS]])
            ident_sb = consts.tile([128, KTS], BF, tag="ident")
            nc.gpsimd.memset(ident_sb, 1.0)
            nc.gpsimd.affine_select(
                out=ident_sb, in_=ident_sb,
                compare_op=mybir.AluOpType.is_ge, fill=0.0,
                base=0, channel_multiplier=-1, pattern=[[1, KTS]])
            nc.gpsimd.affine_select(
                out=ident_sb, in_=ident_sb,
                compare_op=mybir.AluOpType.is_ge, fill=0.0,
                base=0, channel_multiplier=1, pattern=[[-1, KTS]])

            gam_sb = consts.tile([128, D], F32, tag="gam")
            bet_sb = consts.tile([128, D], F32, tag="bet")
            qres_sb4 = consts.tile([128, NQB, D], F32, tag="qres4")

            # ---- persistent activations ----
            qT_sb = [persist.tile([128, L], BF, tag=f"qT{p}", name=f"qT_sb{p}") for p in range(NPAIR)]
            kT_sb = [persist.tile([128, L], BF, tag=f"kT{p}", name=f"kT_sb{p}") for p in range(NPAIR)]
            # V in natural [seq, d] layout, 128 cols per head: col 0 = ones
            # (softmax denominator lands at PSUM partition 0, where GPSIMD
            # partition_broadcast can read it), cols 1:64 zero pad, 64:128 = V
            # (A rows land at partitions 64:128; DVE partition-shifts them to
            # 0:64 during the normalize multiply)
            v128_sb = persist.tile([128, NKT, HL * 128], BF, tag="v128")
            nc.gpsimd.memset(v128_sb, 0.0)
            nc.gpsimd.memset(
                v128_sb.rearrange("p kt (h x) -> p kt h x", x=128)[:, :, :, 0:1], 1.0)
            # normalized attention output A^T: [DH, head, L]
            a4_sb = persist.tile([DH, HL, L], BF, tag="a4", name="a4_sb")
            # gathered A^T after per-qb A2A: [part, qb, ctile, batch, m]
            ob2_sb = persist.tile([128, NQB, 8, 2, DH], BF, tag="ob2", name="ob2_sb")

            in_bq = [dram.tile([L, DH], BF, name=f"in_bq{i}") for i in range(NQB)]
            out_bq = [dram.tile([L, DH], BF, name=f"out_bq{i}") for i in range(NQB)]

            xin_tiles = {}

            def issue_loads(mb, split_in=False):
                m0 = mb * MBS
                xq = xin.tile([128, 8, MBS], BF, tag="xq", name=f"xq{mb}")
                xk = xin.tile([128, 8, MBS], BF, tag="xk", name=f"xk{mb}")
                xv = xin.tile([128, 8, MBS], BF, tag="xv", name=f"xv{mb}")
                xin_tiles[mb] = (xq, xk, xv)
                if split_in:
                    # halve the first transfers so the first matmuls start sooner;
                    # wk/wv load after xq (needed only once Q's matmuls are running)
                    for hf in range(2):
                        nc.sync.dma_start(
                            out=xq[:, 4 * hf:4 * hf + 4, :],
                            in_=qT[512 * hf:512 * hf + 512, m0:m0 + MBS].rearrange(
                                "(t p) m -> p t m", p=128))
                    load_wkv()
                    for src_t, dst in ((kT, xk), (vT, xv)):
                        for hf in range(2):
                            nc.sync.dma_start(
                                out=dst[:, 4 * hf:4 * hf + 4, :],
                                in_=src_t[512 * hf:512 * hf + 512, m0:m0 + MBS].rearrange(
                                    "(t p) m -> p t m", p=128))
                else:
                    nc.sync.dma_start(out=xq, in_=qT[:, m0:m0 + MBS].rearrange("(t p) m -> p t m", p=128))
                    nc.sync.dma_start(out=xk, in_=kT[:, m0:m0 + MBS].rearrange("(t p) m -> p t m", p=128))
                    nc.sync.dma_start(out=xv, in_=vT[:, m0:m0 + MBS].rearrange("(t p) m -> p t m", p=128))

            def proj_compute(mb):
                m0 = mb * MBS
                xq, xk, xv = xin_tiles.pop(mb)
                for p in range(NPAIR):
                    psq = ps_proj.tile([128, MBS], F32, tag="proj")
                    for t in range(8):
                        nc.tensor.matmul(psq[:], wq_sb[:, t, 128 * p:128 * p + 128], xq[:, t, :],
                                         start=(t == 0), stop=(t == 7))
                    if with_biases:
                        nc.vector.tensor_scalar_add(qT_sb[p][:, m0:m0 + MBS], psq[:], bq_sb[:, p:p + 1])
                    else:
                        nc.vector.tensor_copy(qT_sb[p][:, m0:m0 + MBS], psq[:])
                    psk = ps_proj.tile([128, MBS], F32, tag="proj")
                    for t in range(8):
                        nc.tensor.matmul(psk[:], wk_sb[:, t, 128 * p:128 * p + 128], xk[:, t, :],
                                         start=(t == 0), stop=(t == 7))
                    if with_biases:
                        nc.vector.tensor_scalar_add(kT_sb[p][:, m0:m0 + MBS], psk[:], bk_sb[:, p:p + 1])
                    else:
                        nc.vector.tensor_copy(kT_sb[p][:, m0:m0 + MBS], psk[:])
                for ms in range(MBS // 128):
                    mt = mb * (MBS // 128) + ms
                    psv = ps_proj.tile([128, HL * DH], F32, tag="proj")
                    for t in range(8):
                        nc.tensor.matmul(psv[:], xv[:, t, 128 * ms:128 * ms + 128], wv_sb[:, t, :],
                                         start=(t == 0), stop=(t == 7))
                    # write into v128 slots (strided dest); bv is added post-normalize
                    dst = v128_sb[:, mt, :].rearrange("p (h x) -> p h x", x=128)[:, :, 64:128]
                    nc.vector.tensor_copy(dst, psv[:].rearrange("p (h x) -> p h x", x=DH))

            def attn_block(qb, do_a2a=True):
                q0 = qb * QBS
                nkt = 4 * qb + 4
                for p in range(NPAIR):
                    at_e = ps_acc.tile([128, QBS], F32, tag="acc", name="at_e")
                    at_o = ps_acc.tile([128, QBS], F32, tag="acc", name="at_o")
                    es_prev = None
                    for kt in range(nkt):
                        k0 = kt * KTS
                        s_e = ps_s.tile([128, QBS], F32, tag="s")
                        s_o = ps_s.tile([128, QBS], F32, tag="s")
                        d = kt - 4 * qb
                        diag = d >= 0
                        # causally-valid q-slice of this tile (cols < off are fully masked)
                        off = 128 * d if d > 0 else 0
                        nc.tensor.matmul(s_e[:], kT_sb[p][0:64, k0:k0 + KTS],
                                         qT_sb[p][0:64, q0:q0 + QBS],
                                         start=True, stop=not diag)
                        nc.tensor.matmul(s_o[:], kT_sb[p][64:128, k0:k0 + KTS],
                                         qT_sb[p][64:128, q0:q0 + QBS],
                                         start=True, stop=not diag)
                        if diag:  # diagonal-crossing tile: add NEG to masked band
                            b0 = 128 * d
                            for s in (s_e, s_o):
                                nc.tensor.matmul(s[:, b0:b0 + KTS], negT_sb, ident_sb,
                                                 start=False, stop=True)
                        es_e = espool.tile([128, QBS], BF, tag="es_e")
                        es_o = espool.tile([128, QBS], BF, tag="es_o")
                        nc.scalar.activation(out=es_e[:, off:], in_=s_e[:, off:],
                                             func=mybir.ActivationFunctionType.Exp, scale=SCALE)
                        nc.scalar.activation(out=es_o[:, off:], in_=s_o[:, off:],
                                             func=mybir.ActivationFunctionType.Exp, scale=SCALE)
                        # PV for previous kt was already emitted; emit this kt's PV now.
                        # (software pipeline: scores of kt+1 queue ahead of PV of kt on PE)
                        if es_prev is not None:
                            pkt, poff, pes_e, pes_o = es_prev
                            nc.tensor.matmul(at_e[:, poff:], v128_sb[:, pkt, 128 * 2 * p:128 * 2 * p + 128],
                                             pes_e[:, poff:], start=(pkt == 0), stop=False)
                            nc.tensor.matmul(at_o[:, poff:], v128_sb[:, pkt, 128 * (2 * p + 1):128 * (2 * p + 1) + 128],
                                             pes_o[:, poff:], start=(pkt == 0), stop=False)
                        es_prev = (kt, off, es_e, es_o)
                    pkt, poff, pes_e, pes_o = es_prev
                    nc.tensor.matmul(at_e[:, poff:], v128_sb[:, pkt, 128 * 2 * p:128 * 2 * p + 128],
                                     pes_e[:, poff:], start=(pkt == 0), stop=True)
                    nc.tensor.matmul(at_o[:, poff:], v128_sb[:, pkt, 128 * (2 * p + 1):128 * (2 * p + 1) + 128],
                                     pes_o[:, poff:], start=(pkt == 0), stop=True)
                    # normalize: A = A_unnorm * (1/colsum); den sits at PSUM
                    # partition 0 (GPSIMD pbcast reads only partition 0), A rows
                    # at 64:128 (DVE legally partition-shifts 64:128 -> 0:64)
                    for par, at in ((0, at_e), (1, at_o)):
                        h = 2 * p + par
                        rec = work.tile([1, QBS], BF, tag="rec")
                        with nc.allow_low_precision("bf16 softmax reciprocal is within tolerance"):
                            nc.vector.reciprocal(out=rec[:, :], in_=at[0:1, :])
                        bc_sb = work.tile([64, QBS], BF, tag="bc_sb")
                        nc.gpsimd.partition_broadcast(bc_sb[:, :], rec[:, :])
                        nc.vector.tensor_mul(a4_sb[:, h, q0:q0 + QBS], at[64:128, :], bc_sb[:])
                        if with_biases:
                            nc.vector.tensor_scalar_add(a4_sb[:, h, q0:q0 + QBS],
                                                        a4_sb[:, h, q0:q0 + QBS],
                                                        bv_sb[:, h:h + 1])
                if not do_a2a:
                    return
                # A2A input for this q-block: dest chunk j gets A^T cols
                # [512qb+64j : +64] in [(h p) m] row layout
                for h in range(HL):
                    nc.sync.dma_start(
                        out=in_bq[qb].rearrange("(j h p) m -> p h j m", j=NCORES, h=HL, p=DH)[:, h],
                        in_=a4_sb[:, h, q0:q0 + QBS].rearrange("p (j m) -> p j m", j=NCORES))
                nc.gpsimd.collective_compute(
                    "AllToAll", mybir.AluOpType.bypass,
                    ins=[in_bq[qb].opt()], outs=[out_bq[qb].opt()],
                    replica_groups=[list(range(NCORES))])

            def outproj_block(qb):
                # gather so that ctile t has batch0 (rows of src cores 0-3) in
                # cols 0:64 and batch1 in cols 64:128 (same (h,dh) rows); this
                # DMA waits on AllToAll #qb — issued one block late so the wait
                # is (nearly) satisfied at queue-head time
                for b_ in range(2):
                    nc.sync.dma_start(
                        out=ob2_sb[:, qb, :, b_, :],
                        in_=out_bq[qb].rearrange("(b t p) m -> p t b m", b=2, t=8, p=128)[:, :, b_])
                # 128 rows: queries [512qb+64c : +64] of batch0 then batch1
                x_sb = work.tile([128, D], F32, tag="x")
                for nb in range(2):
                    o_ps = ps_out.tile([128, 512], F32, tag="o", name="o_ps")
                    for t in range(8):
                        nc.tensor.matmul(o_ps[:],
                                         ob2_sb[:, qb, t, :, :],
                                         wo_sb[:, t, 512 * nb:512 * nb + 512],
                                         start=(t == 0), stop=(t == 7))
                    nc.vector.tensor_add(x_sb[:, 512 * nb:512 * nb + 512], o_ps[:],
                                         qres_sb4[:, qb, 512 * nb:512 * nb + 512])
                stats = work.tile([128, 2, 6], F32, tag="stats")
                nc.vector.bn_stats(out=stats[:, 0, :], in_=x_sb[:, 0:512])
                nc.vector.bn_stats(out=stats[:, 1, :], in_=x_sb[:, 512:1024])
                mv = work.tile([128, 2], F32, tag="mv")
                nc.vector.bn_aggr(out=mv[:], in_=stats[:])
                # rstd = 1/sqrt(var+eps) = exp(-0.5*log(var+eps)): stays inside
                # the exp/log ACT table set (no table switch)
                rstd = work.tile([128, 1], F32, tag="rstd")
                nc.scalar.activation(out=rstd[:], in_=mv[:, 1:2],
                                     func=mybir.ActivationFunctionType.Ln,
                                     bias=eps_sb[:, 0:1], scale=1.0)
                nc.scalar.activation(out=rstd[:], in_=rstd[:],
                                     func=mybir.ActivationFunctionType.Exp, scale=-0.5)
                y_sb = work.tile([128, D], F32, tag="y")
                nc.vector.tensor_scalar(out=y_sb[:], in0=x_sb[:],
                                        scalar1=mv[:, 0:1], scalar2=rstd[:, 0:1],
                                        op0=mybir.AluOpType.subtract,
                                        op1=mybir.AluOpType.mult)
                nc.vector.scalar_tensor_tensor(out=y_sb[:], in0=y_sb[:], scalar=1.0,
                                               in1=gam_sb[:],
                                               op0=mybir.AluOpType.mult,
                                               op1=mybir.AluOpType.mult)
                nc.vector.tensor_add(y_sb[:], y_sb[:], bet_sb[:])
                nc.sync.dma_start(out=y[128 * qb:128 * qb + 128, :], in_=y_sb[:])

            for _rep in range(reps):
              for i in range(NMB):
                if i == 0:
                    issue_loads(0, split_in=(_rep == 0))
                proj_compute(i)
                if i + 1 < NMB:
                    issue_loads(i + 1)
                if _rep == 0 and i == 1:
                    # E-phase constants: issued after xin(2) so they don't
                    # delay the projection pipeline; needed from out-proj #0 on
                    nc.sync.dma_start(out=wo_sb, in_=wo.rearrange("(t p) n -> p t n", p=128))
                    nc.sync.dma_start(out=qres_sb4,
                                      in_=qresbo.rearrange("(r p) n -> p r n", p=128))
                    nc.sync.dma_start(out=gam_sb, in_=gamma[:, :].to_broadcast([128, D]))
                    nc.sync.dma_start(out=bet_sb, in_=beta[:, :].to_broadcast([128, D]))
                if phases == 'full' and i >= 1:
                    outproj_block(i - 1)
                if phases != 'proj':
                    attn_block(i, do_a2a=(phases in ('a2a', 'full')))
              if phases == 'full':
                  outproj_block(NMB - 1)
    nc.finalize()
    return nc


_CACHE = {}


def _prep_inputs(query, key, value, Wq, bq, Wk, bk, Wv, bv, Wo, bo, gamma, beta):
    """Host-side shard + transpose + cast. Returns per-core in_maps."""
    q32 = np.asarray(query, np.float32)
    qT = [np.ascontiguousarray(q32[b].T).astype(BF16) for b in range(B)]
    kTt = [np.ascontiguousarray(np.asarray(key, np.float32)[b].T).astype(BF16) for b in range(B)]
    vTt = [np.ascontiguousarray(np.asarray(value, np.float32)[b].T).astype(BF16) for b in range(B)]
    Wqb = np.asarray(Wq, np.float32).astype(BF16)
    Wkb = np.asarray(Wk, np.float32).astype(BF16)
    Wvb = np.asarray(Wv, np.float32).astype(BF16)
    Wob = np.ascontiguousarray(np.asarray(Wo, np.float32)).astype(BF16)
    bo32 = np.asarray(bo, np.float32)
    in_maps = []
    for c in range(NCORES):
        b, g = divmod(c, 4)
        sl = slice(HL * DH * g, HL * DH * (g + 1))
        qres = np.concatenate(
            [q32[b_, 512 * qb + 64 * c: 512 * qb + 64 * c + 64] + bo32
             for qb in range(4) for b_ in range(B)], axis=0)
        in_maps.append({
            "qT": qT[b], "kT": kTt[b], "vT": vTt[b],
            "wq": np.ascontiguousarray(Wqb[:, sl]),
            "wk": np.ascontiguousarray(Wkb[:, sl]),
            "wv": np.ascontiguousarray(Wvb[:, sl]),
            "wo": Wob,
            "bq": np.ascontiguousarray(np.asarray(bq, np.float32)[sl]).reshape(HL * DH, 1),
            "bk": np.ascontiguousarray(np.asarray(bk, np.float32)[sl]).reshape(HL * DH, 1),
            "bv": np.ascontiguousarray(np.asarray(bv, np.float32)[sl].reshape(HL, DH).T),
            "qresbo": np.ascontiguousarray(qres, np.float32),
            "gamma": np.asarray(gamma, np.float32).reshape(1, D),
            "beta": np.asarray(beta, np.float32).reshape(1, D),
        })
    return in_maps


def _assemble(results):
    out = np.empty((B, L, D), np.float32)
    for c in range(NCORES):
        yc = results[c]["y"]
        for b_ in range(B):
            for qb in range(4):
                out[b_, 512 * qb + 64 * c: 512 * qb + 64 * c + 64] = \
                    yc[128 * qb + 64 * b_: 128 * qb + 64 * b_ + 64]
    return out


def kernel(**inputs) -> np.ndarray:
    from concourse.bass_utils import run_bass_kernel_spmd
    in_maps = _prep_inputs(
        inputs["query"], inputs["key"], inputs["value"],
        inputs["Wq"], inputs["bq"], inputs["Wk"], inputs["bk"],
        inputs["Wv"], inputs["bv"], inputs["Wo"], inputs["bo"],
        inputs["gamma"], inputs["beta"])
    wb = any(np.any(np.asarray(inputs[k]) != 0) for k in ("bq", "bk", "bv"))
    key = ("nc", wb)
    if key not in _CACHE:
        _CACHE[key] = build_nc(with_biases=wb)
    _CACHE["nc"] = _CACHE[key]
    res = run_bass_kernel_spmd(_CACHE[key], in_maps, core_ids=list(range(NCORES)))
    return _assemble(res.results)


if __name__ == "__main__":
    # quick shape check of the program build
    nc = build_nc()
    n_inst = sum(len(bb.instructions) for f in nc.m.functions for bb in f.blocks)
    print("built ok, instructions:", n_inst)
